# revision 1
# baseline (speedup 1.0000x reference)
"""KAN Fourier-linear kernel for 8 Trainium2 NeuronCores.

y[n,o] = sum_{i,g} C0[o,i,g]*cos(g*x[n,i]) + C1[o,i,g]*sin(g*x[n,i]) + bias[o]

Strategy (data-parallel over n, 4096 rows/core):
  - Features F[k, n] for k=(g,trig,i) computed on-chip:
      v   = int32(x*a_g + C_g)            # gpsimd tensor_scalar (round-to-nearest)
      r_g = x - v*(2pi/g)                 # DVE scalar_tensor_tensor (fp32)
      F   = Sin(scale=g, bias=b)(r_g)     # ACT spline, arg in [-5pi/4, 3pi/4]
    cos and sin share one reduced r_g (phase moved into ACT bias).
  - y.T tile = W.T @ F via PE, K=4096 accumulated in PSUM (bf16 inputs).
  - Host: transpose/shard x, reorder fouriercoeffs, assemble y.
"""
import math
import numpy as np
from contextlib import ExitStack

import concourse.bass as bass
import concourse.mybir as mybir
import concourse.tile as tile
from concourse import bacc
from concourse.bass_utils import run_bass_kernel_spmd

import ml_dtypes

N_CORES = 8
N_TOTAL = 32768
N_SHARD = N_TOTAL // N_CORES        # 4096 rows per core
INDIM = 128
OUTDIM = 256
GRID = 16
K_TOT = 2 * GRID * INDIM            # 4096
SP = 2                              # n-superpasses per core
S = N_SHARD // SP                   # 2048 columns per superpass
CH = 512                            # matmul moving chunk
TWO_PI = 2.0 * math.pi

FP32 = mybir.dt.float32
BF16 = mybir.dt.bfloat16
I32 = mybir.dt.int32


def _g_consts(g: int):
    a = np.float32(g / TWO_PI)
    phat = np.float32(TWO_PI / g)
    m = 2.0 ** math.ceil(math.log2(0.960 * g + 0.14))
    c = np.float32(m + 0.125)
    b_s = np.float32(m * g * float(phat))      # == 2pi*m up to fp32, matched to phat
    b_c = np.float32(float(b_s) + math.pi / 2.0)
    return a, phat, c, b_s, b_c


_CACHED = {}


def _build():
    if "nc" in _CACHED:
        return _CACHED["nc"]
    nc = bacc.Bacc("TRN2", target_bir_lowering=False, debug=False,
                   num_devices=N_CORES)
    xt_d = nc.dram_tensor("xt", [INDIM, N_SHARD], FP32, kind="ExternalInput").ap()
    w_d = nc.dram_tensor("w", [INDIM, 32 * OUTDIM], BF16, kind="ExternalInput").ap()
    bt_d = nc.dram_tensor("bt", [INDIM, 32], FP32, kind="ExternalInput").ap()
    bias_d = nc.dram_tensor("bias", [INDIM, 2], FP32, kind="ExternalInput").ap()
    yt_d = nc.dram_tensor("yt", [OUTDIM, N_SHARD], FP32, kind="ExternalOutput").ap()

    with tile.TileContext(nc) as tc, ExitStack() as ctx:
        cpool = ctx.enter_context(tc.tile_pool(name="const", bufs=1))
        vpool = ctx.enter_context(tc.tile_pool(name="v", bufs=2))
        rpool = ctx.enter_context(tc.tile_pool(name="r", bufs=3))
        fpool = ctx.enter_context(tc.tile_pool(name="f", bufs=4))
        ypool = ctx.enter_context(tc.tile_pool(name="y", bufs=2))
        ppool = ctx.enter_context(tc.tile_pool(name="psum", bufs=1, space="PSUM"))

        xt = cpool.tile([INDIM, N_SHARD], FP32)
        nc.sync.dma_start(xt[:], xt_d[:])
        wt = cpool.tile([INDIM, 32 * OUTDIM], BF16)
        nc.sync.dma_start(wt[:], w_d[:])
        bt = cpool.tile([INDIM, 32], FP32)
        nc.sync.dma_start(bt[:], bt_d[:])
        bias = cpool.tile([INDIM, 2], FP32)
        nc.sync.dma_start(bias[:], bias_d[:])

        for sp in range(SP):
            xs = xt[:, sp * S:(sp + 1) * S]
            psum0 = ppool.tile([128, S], FP32, tag="p0")
            psum1 = ppool.tile([128, S], FP32, tag="p1")
            psums = [psum0, psum1]
            for gi in range(GRID):
                g = gi + 1
                a, phat, c, b_s, b_c = _g_consts(g)
                v = vpool.tile([INDIM, S], I32, tag="v")
                nc.gpsimd.tensor_scalar(v[:], xs, float(a), float(c),
                                        mybir.AluOpType.mult, mybir.AluOpType.add)
                r = rpool.tile([INDIM, S], FP32, tag="r")
                nc.vector.scalar_tensor_tensor(r[:], v[:], float(-phat), xs,
                                               mybir.AluOpType.mult,
                                               mybir.AluOpType.add)
                for t in range(2):            # 0=cos, 1=sin
                    kt = 2 * gi + t
                    f = fpool.tile([INDIM, S], BF16, tag="f")
                    nc.scalar.activation(f[:], r[:],
                                         mybir.ActivationFunctionType.Sin,
                                         bias=bt[:, kt:kt + 1], scale=float(g))
                    for oh in range(2):
                        lhsT = wt[:, kt * OUTDIM + oh * 128:
                                  kt * OUTDIM + oh * 128 + 128]
                        for chi in range(S // CH):
                            nc.tensor.matmul(
                                psums[oh][:, chi * CH:(chi + 1) * CH],
                                lhsT, f[:, chi * CH:(chi + 1) * CH],
                                start=(kt == 0), stop=(kt == 31),
                            )
            for oh in range(2):
                y = ypool.tile([128, S], FP32, tag=f"y{oh}")
                nc.vector.tensor_scalar(y[:], psums[oh][:], bias[:, oh:oh + 1],
                                        None, mybir.AluOpType.add)
                nc.sync.dma_start(
                    yt_d[oh * 128:(oh + 1) * 128, sp * S:(sp + 1) * S], y[:])

    nc.compile()
    _CACHED["nc"] = nc
    return nc


def _prep_inputs(x: np.ndarray, fouriercoeffs: np.ndarray, bias: np.ndarray):
    xt = np.ascontiguousarray(x.astype(np.float32, copy=False).T)  # (128, 32768)
    # W2[k, o], k = (g-1)*256 + t*128 + i
    w2 = np.ascontiguousarray(
        fouriercoeffs.astype(np.float32, copy=False).transpose(3, 0, 2, 1)
    ).reshape(K_TOT, OUTDIM)
    w_sb = np.ascontiguousarray(
        w2.reshape(32, 128, OUTDIM).transpose(1, 0, 2).reshape(128, 32 * OUTDIM)
    ).astype(ml_dtypes.bfloat16)
    bvals = np.empty(32, np.float32)
    for gi in range(GRID):
        _, _, _, b_s, b_c = _g_consts(gi + 1)
        bvals[2 * gi] = b_c
        bvals[2 * gi + 1] = b_s
    bt = np.tile(bvals[None, :], (INDIM, 1)).astype(np.float32)
    bias_sb = np.ascontiguousarray(
        bias.reshape(2, 128).T.astype(np.float32))      # (128, 2)
    return xt, w_sb, bt, bias_sb


def kernel(x: np.ndarray, fouriercoeffs: np.ndarray, bias: np.ndarray,
           _trace: bool = False):
    x = np.asarray(x)
    fouriercoeffs = np.asarray(fouriercoeffs)
    bias = np.asarray(bias)
    orig_shape = x.shape
    x2 = x.reshape(-1, INDIM)
    assert x2.shape == (N_TOTAL, INDIM), x2.shape

    nc = _build()
    xt, w_sb, bt, bias_sb = _prep_inputs(x2, fouriercoeffs, bias)
    in_maps = []
    for c in range(N_CORES):
        in_maps.append({
            "xt": np.ascontiguousarray(xt[:, c * N_SHARD:(c + 1) * N_SHARD]),
            "w": w_sb,
            "bt": bt,
            "bias": bias_sb,
        })
    res = run_bass_kernel_spmd(nc, in_maps, list(range(N_CORES)),
                               trace=_trace)
    yt = np.concatenate([res.results[c]["yt"] for c in range(N_CORES)], axis=1)
    y = np.ascontiguousarray(yt.T).astype(np.float32)
    if _trace:
        kernel._last_result = res
    return y.reshape(*orig_shape[:-1], OUTDIM)



# revision 2
# speedup vs baseline: 1.1290x; 1.1290x over previous
"""KAN Fourier-linear kernel for 8 Trainium2 NeuronCores.

y[n,o] = sum_{i,g} C0[o,i,g]*cos(g*x[n,i]) + C1[o,i,g]*sin(g*x[n,i]) + bias[o]

Strategy (data-parallel over n, 4096 rows/core, 2 superpasses of 2048):
  - Seeds g=1..8 computed exactly:
      v   = rne(x*(g/2pi) + 1/8)          (DVE tensor_scalar fp32->int32)
      r   = x - v*(2pi/g)                 (DVE scalar_tensor_tensor fp32)
      S_g = Sin(scale=g, bias=0)(r)       = sin(g*x)   (ACT, arg in [-3.93, 2.36])
      C_g = Sin(scale=g, bias=pi/2)(r)    = cos(g*x)   (ACT, arg in [-2.36, 3.93])
  - g=9..16 as product features on DVE (TT bf16, 2x mode):
      Q_r = S_8*C_r, P_r = C_8*C_r  (r=1..8)
    using sin((8+r)x) = 2*Q_r - sin((8-r)x), cos((8+r)x) = 2*P_r - cos((8-r)x);
    the linear reconstruction is absorbed into the weights on the host.
  - y.T = W'.T @ F via PE (bf16, K=4096 accumulated in PSUM over 32 k-tiles).
  - PSUM drained on ACT (Identity + per-partition bias).
  - Host: transpose/shard x, build W' from fouriercoeffs (fp64), assemble y.
"""
import math
import numpy as np
from contextlib import ExitStack

import concourse.bass as bass
import concourse.mybir as mybir
import concourse.tile as tile
from concourse import bacc
from concourse.bass_utils import run_bass_kernel_spmd

import ml_dtypes

N_CORES = 8
N_TOTAL = 32768
N_SHARD = N_TOTAL // N_CORES        # 4096 rows per core
INDIM = 128
OUTDIM = 256
GRID = 16
NFEAT = 32                          # 2*GRID features per i
SP = 2                              # n-superpasses per core
S = N_SHARD // SP                   # 2048 columns per superpass
CH = 512                            # matmul moving chunk (PSUM bank)
TWO_PI = 2.0 * math.pi

FP32 = mybir.dt.float32
BF16 = mybir.dt.bfloat16
I32 = mybir.dt.int32

_CACHED = {}


def _build():
    if "nc" in _CACHED:
        return _CACHED["nc"]
    nc = bacc.Bacc("TRN2", target_bir_lowering=False, debug=False,
                   num_devices=N_CORES)
    xt_d = nc.dram_tensor("xt", [INDIM, N_SHARD], FP32, kind="ExternalInput").ap()
    w_d = nc.dram_tensor("w", [INDIM, NFEAT * OUTDIM], BF16,
                         kind="ExternalInput").ap()
    bt_d = nc.dram_tensor("bt", [INDIM, 2], FP32, kind="ExternalInput").ap()
    bias_d = nc.dram_tensor("bias", [INDIM, 2], FP32, kind="ExternalInput").ap()
    yt_d = nc.dram_tensor("yt", [OUTDIM, N_SHARD], FP32, kind="ExternalOutput").ap()

    with tile.TileContext(nc) as tc, ExitStack() as ctx:
        cpool = ctx.enter_context(tc.tile_pool(name="const", bufs=1))
        vpool = ctx.enter_context(tc.tile_pool(name="v", bufs=2))
        rpool = ctx.enter_context(tc.tile_pool(name="r", bufs=2))
        seedpool = ctx.enter_context(tc.tile_pool(name="seed", bufs=1))
        prodpool = ctx.enter_context(tc.tile_pool(name="prod", bufs=4))
        ypool = ctx.enter_context(tc.tile_pool(name="y", bufs=2))
        ppool = ctx.enter_context(tc.tile_pool(name="psum", bufs=1, space="PSUM"))

        xt = cpool.tile([INDIM, N_SHARD], FP32)
        nc.sync.dma_start(xt[:], xt_d[:])
        wt = cpool.tile([INDIM, NFEAT * OUTDIM], BF16)
        nc.sync.dma_start(wt[:], w_d[:])
        bt = cpool.tile([INDIM, 2], FP32)
        nc.sync.dma_start(bt[:], bt_d[:])
        bias = cpool.tile([INDIM, 2], FP32)
        nc.sync.dma_start(bias[:], bias_d[:])

        def mm_feature(kt, f, psums):
            """Accumulate feature tile f (k-tile kt) into both output halves."""
            for oh in range(2):
                lhsT = wt[:, kt * OUTDIM + oh * 128:kt * OUTDIM + oh * 128 + 128]
                for chi in range(S // CH):
                    nc.tensor.matmul(
                        psums[oh][:, chi * CH:(chi + 1) * CH],
                        lhsT, f[:, chi * CH:(chi + 1) * CH],
                        start=(kt == 0), stop=(kt == NFEAT - 1),
                    )

        for sp in range(SP):
            xs = xt[:, sp * S:(sp + 1) * S]
            psum0 = ppool.tile([128, S], FP32, tag="p0")
            psum1 = ppool.tile([128, S], FP32, tag="p1")
            psums = [psum0, psum1]

            seeds_S = [None] * 9    # 1-indexed by g
            seeds_C = [None] * 9
            for g in range(1, 9):
                a = np.float32(g / TWO_PI)
                p = np.float32(TWO_PI / g)
                v = vpool.tile([INDIM, S], I32, tag="v")
                nc.vector.tensor_scalar(v[:], xs, float(a), 0.125,
                                        mybir.AluOpType.mult,
                                        mybir.AluOpType.add)
                r = rpool.tile([INDIM, S], FP32, tag="r")
                nc.vector.scalar_tensor_tensor(r[:], v[:], float(-p), xs,
                                               mybir.AluOpType.mult,
                                               mybir.AluOpType.add)
                sg = seedpool.tile([INDIM, S], BF16, tag=f"S{g}")
                nc.scalar.activation(sg[:], r[:],
                                     mybir.ActivationFunctionType.Sin,
                                     bias=bt[:, 0:1], scale=float(g))
                cg = seedpool.tile([INDIM, S], BF16, tag=f"C{g}")
                nc.scalar.activation(cg[:], r[:],
                                     mybir.ActivationFunctionType.Sin,
                                     bias=bt[:, 1:2], scale=float(g))
                seeds_S[g] = sg
                seeds_C[g] = cg
                mm_feature(2 * (g - 1), sg, psums)
                mm_feature(2 * (g - 1) + 1, cg, psums)

            for rr in range(1, 9):
                q = prodpool.tile([INDIM, S], BF16, tag="q")
                nc.vector.tensor_tensor(q[:], seeds_S[8][:], seeds_C[rr][:],
                                        mybir.AluOpType.mult)
                mm_feature(16 + 2 * (rr - 1), q, psums)
                pr = prodpool.tile([INDIM, S], BF16, tag="p")
                nc.vector.tensor_tensor(pr[:], seeds_C[8][:], seeds_C[rr][:],
                                        mybir.AluOpType.mult)
                mm_feature(16 + 2 * (rr - 1) + 1, pr, psums)

            for oh in range(2):
                y = ypool.tile([128, S], FP32, tag=f"y{oh}")
                nc.scalar.activation(y[:], psums[oh][:],
                                     mybir.ActivationFunctionType.Identity,
                                     bias=bias[:, oh:oh + 1], scale=1.0)
                nc.sync.dma_start(
                    yt_d[oh * 128:(oh + 1) * 128, sp * S:(sp + 1) * S], y[:])

    nc.compile()
    _CACHED["nc"] = nc
    return nc


def _prep_inputs(x: np.ndarray, fouriercoeffs: np.ndarray, bias: np.ndarray):
    xt = np.ascontiguousarray(x.astype(np.float32, copy=False).T)  # (128, 32768)

    fc = fouriercoeffs.astype(np.float64, copy=False)
    c_cos = fc[0]                     # (256 o, 128 i, 16 g): cos coeffs
    c_sin = fc[1]                     # sin coeffs

    # Feature k-tile order:
    #   kt = 2(g-1)   : S_g = sin(gx)          (g = 1..8)
    #   kt = 2(g-1)+1 : C_g = cos(gx)
    #   kt = 16+2(r-1)   : Q_r = S_8*C_r       (r = 1..8)
    #   kt = 16+2(r-1)+1 : P_r = C_8*C_r
    # sin((8+r)x) = 2 Q_r - sin((8-r)x)   [sin(0x) = 0]
    # cos((8+r)x) = 2 P_r - cos((8-r)x)   [cos(0x) = 1 -> bias]
    wf = np.zeros((NFEAT, OUTDIM, INDIM), np.float64)
    for g in range(1, 9):
        wf[2 * (g - 1)] = c_sin[:, :, g - 1]
        wf[2 * (g - 1) + 1] = c_cos[:, :, g - 1]
    for r in range(1, 9):
        gs = 8 + r
        wf[16 + 2 * (r - 1)] += 2.0 * c_sin[:, :, gs - 1]
        wf[16 + 2 * (r - 1) + 1] += 2.0 * c_cos[:, :, gs - 1]
        if r < 8:
            wf[2 * (8 - r - 1)] -= c_sin[:, :, gs - 1]        # S_{8-r}
            wf[2 * (8 - r - 1) + 1] -= c_cos[:, :, gs - 1]    # C_{8-r}
    bias_eff = bias.astype(np.float64).reshape(OUTDIM).copy()
    bias_eff -= c_cos[:, :, 15].sum(axis=1)                   # cos(0x)*W_c16

    # SBUF weight tile: [i, kt*256 + o] bf16
    w_sb = np.ascontiguousarray(
        wf.transpose(2, 0, 1).reshape(INDIM, NFEAT * OUTDIM)
    ).astype(ml_dtypes.bfloat16)

    bt = np.tile(np.array([[0.0, math.pi / 2.0]], dtype=np.float32), (INDIM, 1))
    bias_sb = np.ascontiguousarray(
        bias_eff.reshape(2, 128).T.astype(np.float32))        # (128, 2)
    return xt, w_sb, bt, bias_sb


def kernel(x: np.ndarray, fouriercoeffs: np.ndarray, bias: np.ndarray,
           _trace: bool = False):
    x = np.asarray(x)
    fouriercoeffs = np.asarray(fouriercoeffs)
    bias = np.asarray(bias)
    orig_shape = x.shape
    x2 = x.reshape(-1, INDIM)
    assert x2.shape == (N_TOTAL, INDIM), x2.shape

    nc = _build()
    xt, w_sb, bt, bias_sb = _prep_inputs(x2, fouriercoeffs, bias)
    in_maps = []
    for c in range(N_CORES):
        in_maps.append({
            "xt": np.ascontiguousarray(xt[:, c * N_SHARD:(c + 1) * N_SHARD]),
            "w": w_sb,
            "bt": bt,
            "bias": bias_sb,
        })
    res = run_bass_kernel_spmd(nc, in_maps, list(range(N_CORES)),
                               trace=_trace)
    yt = np.concatenate([res.results[c]["yt"] for c in range(N_CORES)], axis=1)
    y = np.ascontiguousarray(yt.T).astype(np.float32)
    if _trace:
        kernel._last_result = res
    return y.reshape(*orig_shape[:-1], OUTDIM)


# revision 7
# speedup vs baseline: 1.1380x; 1.0080x over previous
"""KAN Fourier-linear kernel for 8 Trainium2 NeuronCores.

y[n,o] = sum_{i,g} C0[o,i,g]*cos(g*x[n,i]) + C1[o,i,g]*sin(g*x[n,i]) + bias[o]

Strategy (data-parallel over n, 4096 rows/core, 2 superpasses of 2048):
  - Seeds g=1..8 computed exactly:
      v   = rne(x*(g/2pi) + 1/8)          (DVE tensor_scalar fp32->int32)
      r   = x - v*(2pi/g)                 (DVE scalar_tensor_tensor fp32)
      S_g = Sin(scale=g, bias=0)(r)       = sin(g*x)   (ACT, arg in [-3.93, 2.36])
      C_g = Sin(scale=g, bias=pi/2)(r)    = cos(g*x)   (ACT, arg in [-2.36, 3.93])
  - g=9..16 as product features on DVE (TT bf16, 2x mode):
      Q_r = S_8*C_r, P_r = C_8*C_r  (r=1..8)
    using sin((8+r)x) = 2*Q_r - sin((8-r)x), cos((8+r)x) = 2*P_r - cos((8-r)x);
    the linear reconstruction is absorbed into the weights on the host.
  - y.T = W'.T @ F via PE (bf16, K=4096 accumulated in PSUM over 32 k-tiles).
  - PSUM drained on ACT (Identity + per-partition bias).
  - Host: transpose/shard x, build W' from fouriercoeffs (fp64), assemble y.
"""
import math
import numpy as np
from contextlib import ExitStack

import concourse.bass as bass
import concourse.mybir as mybir
import concourse.tile as tile
from concourse import bacc
from concourse.bass_utils import run_bass_kernel_spmd

import ml_dtypes

N_CORES = 8
N_TOTAL = 32768
N_SHARD = N_TOTAL // N_CORES        # 4096 rows per core
INDIM = 128
OUTDIM = 256
GRID = 16
NFEAT = 32                          # 2*GRID features per i
SP = 2                              # n-superpasses per core
S = N_SHARD // SP                   # 2048 columns per superpass
CH = 512                            # matmul moving chunk (PSUM bank)
TWO_PI = 2.0 * math.pi

FP32 = mybir.dt.float32
BF16 = mybir.dt.bfloat16
I32 = mybir.dt.int32

_CACHED = {}


def _build():
    if "nc" in _CACHED:
        return _CACHED["nc"]
    nc = bacc.Bacc("TRN2", target_bir_lowering=False, debug=False,
                   num_devices=N_CORES)
    xt_d = nc.dram_tensor("xt", [INDIM, N_SHARD], FP32, kind="ExternalInput").ap()
    w_d = nc.dram_tensor("w", [INDIM, NFEAT * OUTDIM], BF16,
                         kind="ExternalInput").ap()
    bt_d = nc.dram_tensor("bt", [INDIM, 2], FP32, kind="ExternalInput").ap()
    bias_d = nc.dram_tensor("bias", [INDIM, 2], FP32, kind="ExternalInput").ap()
    yt_d = nc.dram_tensor("yt", [OUTDIM, N_SHARD], FP32, kind="ExternalOutput").ap()

    with tile.TileContext(nc) as tc, ExitStack() as ctx:
        cpool = ctx.enter_context(tc.tile_pool(name="const", bufs=1))
        vpool = ctx.enter_context(tc.tile_pool(name="v", bufs=1))
        rpool = ctx.enter_context(tc.tile_pool(name="r", bufs=2))
        spool_ = ctx.enter_context(tc.tile_pool(name="seedS", bufs=1))
        cseed = ctx.enter_context(tc.tile_pool(name="seedC", bufs=2))
        prodpool = ctx.enter_context(tc.tile_pool(name="prod", bufs=4))
        ypool = ctx.enter_context(tc.tile_pool(name="y", bufs=1))
        ppool = ctx.enter_context(tc.tile_pool(name="psum", bufs=1, space="PSUM"))

        xt = cpool.tile([INDIM, N_SHARD], FP32)
        for xc in range(4):
            q = N_SHARD // 4
            nc.sync.dma_start(xt[:, xc * q:(xc + 1) * q],
                              xt_d[:, xc * q:(xc + 1) * q])
        bt = cpool.tile([INDIM, 2], FP32)
        nc.sync.dma_start(bt[:], bt_d[:])
        bias = cpool.tile([INDIM, 2], FP32)
        nc.sync.dma_start(bias[:], bias_d[:])
        wt = cpool.tile([INDIM, NFEAT * OUTDIM], BF16)
        nc.sync.dma_start(wt[:], w_d[:])

        def mm_feature(kt, f, psums):
            """Accumulate feature tile f (k-tile kt) into both output halves."""
            for oh in range(2):
                lhsT = wt[:, kt * OUTDIM + oh * 128:kt * OUTDIM + oh * 128 + 128]
                for chi in range(S // CH):
                    nc.tensor.matmul(
                        psums[oh][:, chi * CH:(chi + 1) * CH],
                        lhsT, f[:, chi * CH:(chi + 1) * CH],
                        start=(kt == 0), stop=(kt == NFEAT - 1),
                    )

        for sp in range(SP):
            xs = xt[:, sp * S:(sp + 1) * S]
            psum0 = ppool.tile([128, S], FP32, tag="p0")
            psum1 = ppool.tile([128, S], FP32, tag="p1")
            psums = [psum0, psum1]

            seeds_S = [None] * 9    # 1-indexed by g
            seeds_C = [None] * 9
            for g in range(1, 9):
                a = np.float32(g / TWO_PI)
                p = np.float32(TWO_PI / g)
                v = vpool.tile([INDIM, S], I32, tag="v")
                nc.vector.tensor_scalar(v[:], xs, float(a), 0.125,
                                        mybir.AluOpType.mult,
                                        mybir.AluOpType.add)
                r = rpool.tile([INDIM, S], FP32, tag="r")
                nc.vector.scalar_tensor_tensor(r[:], v[:], float(-p), xs,
                                               mybir.AluOpType.mult,
                                               mybir.AluOpType.add)
                sg = spool_.tile([INDIM, S], BF16, tag=f"S{g}")
                nc.scalar.activation(sg[:], r[:],
                                     mybir.ActivationFunctionType.Sin,
                                     bias=bt[:, 0:1], scale=float(g))
                cg = cseed.tile([INDIM, S], BF16, tag=f"C{g}")
                nc.scalar.activation(cg[:], r[:],
                                     mybir.ActivationFunctionType.Sin,
                                     bias=bt[:, 1:2], scale=float(g))
                seeds_S[g] = sg
                seeds_C[g] = cg
                mm_feature(2 * (g - 1), sg, psums)
                mm_feature(2 * (g - 1) + 1, cg, psums)

            def drain(oh, nchunks=2):
                dc = S // nchunks
                for ci in range(nchunks):
                    y = ypool.tile([128, dc], FP32, tag=f"y{oh}{ci}")
                    nc.scalar.activation(y[:], psums[oh][:, ci * dc:(ci + 1) * dc],
                                         mybir.ActivationFunctionType.Identity,
                                         bias=bias[:, oh:oh + 1], scale=1.0)
                    nc.sync.dma_start(
                        yt_d[oh * 128:(oh + 1) * 128,
                             sp * S + ci * dc:sp * S + (ci + 1) * dc], y[:])

            for rr in range(1, 9):
                q = prodpool.tile([INDIM, S], BF16, tag="q")
                nc.vector.tensor_tensor(q[:], seeds_S[8][:], seeds_C[rr][:],
                                        mybir.AluOpType.mult)
                mm_feature(16 + 2 * (rr - 1), q, psums)
                pr = prodpool.tile([INDIM, S], BF16, tag="p")
                nc.vector.tensor_tensor(pr[:], seeds_C[8][:], seeds_C[rr][:],
                                        mybir.AluOpType.mult)
                if rr < 8:
                    mm_feature(16 + 2 * (rr - 1) + 1, pr, psums)
                else:
                    # last k-tile: finish oh0, drain it while oh1 finishes
                    kt = 31
                    for oh in range(2):
                        lhsT = wt[:, kt * OUTDIM + oh * 128:
                                  kt * OUTDIM + oh * 128 + 128]
                        for chi in range(S // CH):
                            nc.tensor.matmul(
                                psums[oh][:, chi * CH:(chi + 1) * CH],
                                lhsT, pr[:, chi * CH:(chi + 1) * CH],
                                start=False, stop=True,
                            )
                        drain(oh)

    nc.compile()
    _CACHED["nc"] = nc
    return nc


def _prep_inputs(x: np.ndarray, fouriercoeffs: np.ndarray, bias: np.ndarray):
    xt = np.ascontiguousarray(x.astype(np.float32, copy=False).T)  # (128, 32768)

    fc = fouriercoeffs.astype(np.float64, copy=False)
    c_cos = fc[0]                     # (256 o, 128 i, 16 g): cos coeffs
    c_sin = fc[1]                     # sin coeffs

    # Feature k-tile order:
    #   kt = 2(g-1)   : S_g = sin(gx)          (g = 1..8)
    #   kt = 2(g-1)+1 : C_g = cos(gx)
    #   kt = 16+2(r-1)   : Q_r = S_8*C_r       (r = 1..8)
    #   kt = 16+2(r-1)+1 : P_r = C_8*C_r
    # sin((8+r)x) = 2 Q_r - sin((8-r)x)   [sin(0x) = 0]
    # cos((8+r)x) = 2 P_r - cos((8-r)x)   [cos(0x) = 1 -> bias]
    wf = np.zeros((NFEAT, OUTDIM, INDIM), np.float64)
    for g in range(1, 9):
        wf[2 * (g - 1)] = c_sin[:, :, g - 1]
        wf[2 * (g - 1) + 1] = c_cos[:, :, g - 1]
    for r in range(1, 9):
        gs = 8 + r
        wf[16 + 2 * (r - 1)] += 2.0 * c_sin[:, :, gs - 1]
        wf[16 + 2 * (r - 1) + 1] += 2.0 * c_cos[:, :, gs - 1]
        if r < 8:
            wf[2 * (8 - r - 1)] -= c_sin[:, :, gs - 1]        # S_{8-r}
            wf[2 * (8 - r - 1) + 1] -= c_cos[:, :, gs - 1]    # C_{8-r}
    bias_eff = bias.astype(np.float64).reshape(OUTDIM).copy()
    bias_eff -= c_cos[:, :, 15].sum(axis=1)                   # cos(0x)*W_c16

    # SBUF weight tile: [i, kt*256 + o] bf16
    w_sb = np.ascontiguousarray(
        wf.transpose(2, 0, 1).reshape(INDIM, NFEAT * OUTDIM)
    ).astype(ml_dtypes.bfloat16)

    bt = np.tile(np.array([[0.0, math.pi / 2.0]], dtype=np.float32), (INDIM, 1))
    bias_sb = np.ascontiguousarray(
        bias_eff.reshape(2, 128).T.astype(np.float32))        # (128, 2)
    return xt, w_sb, bt, bias_sb


def kernel(x: np.ndarray, fouriercoeffs: np.ndarray, bias: np.ndarray,
           _trace: bool = False):
    x = np.asarray(x)
    fouriercoeffs = np.asarray(fouriercoeffs)
    bias = np.asarray(bias)
    orig_shape = x.shape
    x2 = x.reshape(-1, INDIM)
    assert x2.shape == (N_TOTAL, INDIM), x2.shape

    nc = _build()
    xt, w_sb, bt, bias_sb = _prep_inputs(x2, fouriercoeffs, bias)
    in_maps = []
    for c in range(N_CORES):
        in_maps.append({
            "xt": np.ascontiguousarray(xt[:, c * N_SHARD:(c + 1) * N_SHARD]),
            "w": w_sb,
            "bt": bt,
            "bias": bias_sb,
        })
    res = run_bass_kernel_spmd(nc, in_maps, list(range(N_CORES)),
                               trace=_trace)
    yt = np.concatenate([res.results[c]["yt"] for c in range(N_CORES)], axis=1)
    y = np.ascontiguousarray(yt.T).astype(np.float32)
    if _trace:
        kernel._last_result = res
    return y.reshape(*orig_shape[:-1], OUTDIM)
